# revision 37
# baseline (speedup 1.0000x reference)
"""Trainium2 Bass kernel for nn_Balancer_10660108829428.

Computes (total, fg_loss, bg_loss) for a fg/bg-weighted loss balancer:
  fg_mask[b,h,w] = any of 48 boxes covers pixel (h,w)
  fg_loss = 13 * sum(loss * fg) / (B*H*W)
  bg_loss = sum(loss * ~fg) / (B*H*W)
  total   = fg_loss + bg_loss

Strategy: data-parallel over B across 8 NeuronCores (8 batches each).
Per core each batch's mask is rasterized as a rank-48 matmul
(row_inT @ col_in) on the TensorEngine; fused DVE scalar_tensor_tensor
ops compute (mask > 0) * loss with a free per-partition row-sum
accumulator.

The kernel is DMA-bound (15 MB of loss per core at 360 GB/s in the
cost model, serialized on the single DMA_ENGINES device), so the whole
game is the head (first-DMA issue latency, ~2.0us) and the tail after
the last loss byte:
  last byte -> +900ns DMA-sem prop to the consumer -> last compute op
  -> out-DMA issue (seq+HWDGE+DGE ~1.3us) -> transfer -> +900ns sem
  prop -> ~0.6us barrier epilogue.
Tail design (engine assignments from an offline makespan search over
the cost model: DVE stt 1.042ns/col+61, Pool product 1.98ns/col+95,
Act sall 0.833ns/col+372+187 accum-read, PE [1,n>=256] f32r sall
107ns/256 cols, every op gated at its piece's arrival + ~947ns):
 - pairs 0-2 run the proven line-rate pipeline: per chunk one 2-batch
   DMA, Act all-sum, two PE count matmuls + DVE stt fg ops; three
   pair-3 fg masks (hc0q1, t6, t7) are precomputed as fp16 Sign tiles
   in the count rotation (sign(cnt) == fg since counts >= 0);
 - pair-3 hc0/hc1: per-batch DMAs; fg on DVE from just-in-time PSUM
   counts except hc0q1 which goes Pool-product -> Act-sall (GPSIMD has
   no accumulator and cannot read PSUM); all-sums on PE -> ps_a,
   closed mid-stream by Act;
 - tail (120 rows): pieces b6 512/224/224/288, b7 736/288/224 (b6's
   middle 448 split into two DMAs purely for finer semaphore grain —
   it pulls two DVE ops into an idle gap); fg on DVE except b6[0:512]
   and b7[0:384] which go through Pool-product -> Act-sall chains;
   all-sums as PE [1,>=256] f32r chunks -> ps_b except b7[992:1248]
   which DVE sums right after its last fg (it would otherwise gate the
   Act close at last-byte + ~1.9us); the Act close of ps_b and DVE's
   last op both land ~1.6us after the last byte, balanced within
   ~100ns, right at the out-DMA issue path's floor;
 - the accumulator is split in two: the bulk [128,42] tile (early
   writers) DMAs out ~1.2us before the end and a tiny [128,6] tile
   holding only the six late-written columns rides a second DMA whose
   transfer runs at the 7ns/descriptor floor — the two issue windows
   don't overlap, so HWDGE never serializes them.
The final reduction of the [128, OUTC] accumulator tile is done on the
host (pure gather arithmetic over named column groups).

Box membership avoids floor/ceil entirely: for integer h,
  h >= floor(v1)  <=>  h > v1 - 1      and      h < ceil(v2)  <=>  h < v2.
"""

import numpy as np

import concourse.bacc as bacc
import concourse.mybir as mybir
import concourse.tile as tile
from concourse.bass_utils import run_bass_kernel_spmd

B, H, W, N = 64, 376, 1248, 48
N_CORES = 8
BPC = B // N_CORES          # batches per core
PAIRS = BPC // 2            # batch pairs per core (masks built 2 batches at a time)
FG_WEIGHT = 13.0
H_CHUNKS = [(0, 128), (128, 128), (256, H - 256)]  # (h0, hsz)
F32 = mybir.dt.float32
F32R = mybir.dt.float32r
BF16 = mybir.dt.bfloat16
FP16 = mybir.dt.float16

# accumulator column layout (host sums these)
#  0..17  : fg partials pairs 0-2 (p*6 + hc*2 + q)
#  18..21 : pair-3 hc0q0, hc0q1, hc1q0, hc1q1 fg
#  22..28 : pair-3 tail piece fg
#  29     : b7A cols [0:384] fg (Pool-product -> Act-sall chain)
#  30..38 : all-pixel partials pairs 0-2 (Act salls, 30 + p*3 + hc)
#  39     : ps_a close (pair-3 hc0/hc1 all-sum)
#  40     : ps_b close (pair-3 tail all-sum, minus b7 [992:1248])
#  41     : b7 [992:1248] all-sum (DVE, after its last fg op)
OUTC = 42
FG_LO, FG_HI = 0, 30
ALL_LO, ALL_HI = 30, 42

# tail piece layout: (batch-in-pair q, c0, csz, fg engine, acc col)
# "C" = Pool tensor_tensor product -> Act sall chain (GPSIMD cannot do
# stt-with-accum, and this keeps DVE free for the late pieces).
TAIL_PIECES = [
    (0, 0, 512, "C", 22),
    (1, 0, 736, "D", 23),
    (0, 512, 224, "D", 24),
    (0, 736, 224, "D", 25),
    (0, 960, 288, "D", 26),
    (1, 736, 288, "D", 27),
    (1, 1024, 224, "D", 28),
]
# all-sum sub-chunks per batch for the tail (PE f32r fast path needs
# n >= 256; the leftover 224 chunk goes first where it is gated early)
TAIL_SALL_CHUNKS = [(0, 224), (224, 256), (480, 256), (736, 256), (992, 256)]

_NC_CACHE = None


def _build_nc():
    # Bacc (not bass.Bass): its finalize() runs the TRN2 wait-legalization
    # passes (move_matmul_waits_to_ldweights / generate_event_semaphores) —
    # the ISA allows only one semaphore wait per instruction.
    nc = bacc.Bacc("TRN2")
    loss_d = nc.dram_tensor("loss", [BPC, H, W], F32, kind="ExternalInput")
    # boxes arrive host-transposed to the on-chip (q*64+n, 4*p+c) layout so
    # the DMA descriptors are contiguous 64 B runs instead of 16 B scatters
    boxes_d = nc.dram_tensor("boxes", [2, N, 4 * PAIRS], F32, kind="ExternalInput")
    out_d = nc.dram_tensor("out", [128, OUTC], F32, kind="ExternalOutput")
    # late-written accumulator columns go to a separate tiny tile/output so
    # the bulk out-DMA can issue ~1.2us earlier and the final DMA's transfer
    # shrinks to the 7ns/descriptor floor (56ns vs 119ns)
    out2_d = nc.dram_tensor("out2", [128, 6], F32, kind="ExternalOutput")

    AX = mybir.AxisListType
    OP = mybir.AluOpType
    AF = mybir.ActivationFunctionType

    with tile.TileContext(nc) as tc:
        with (
            tc.tile_pool(name="singles", bufs=1) as singles,
            tc.tile_pool(name="masks", bufs=4) as masks,
            tc.tile_pool(name="ltiles", bufs=7) as ltiles,
            tc.tile_pool(name="scratch", bufs=2) as scratch,
            tc.tile_pool(name="sfpool", bufs=4) as sfpool,
            tc.tile_pool(name="cpsum", bufs=2, space="PSUM") as cpsum,
            tc.tile_pool(name="spsum", bufs=1, space="PSUM") as spsum,
        ):
            # --- constants ---
            iota_i = singles.tile([128, W], mybir.dt.int32)
            nc.gpsimd.iota(iota_i, pattern=[[1, W]], base=0, channel_multiplier=0)
            # fp16 iota: integers <= 2048 are exact, and 2-byte operands let
            # the DVE mask compares run in 4x mode
            iota_f = singles.tile([128, W], FP16)
            nc.vector.tensor_copy(iota_f, iota_i)
            ones = singles.tile([128, 1], F32)
            nc.vector.memset(ones, 1.0)
            # f32r view for the PE all-sum matmuls: the BIR verifier requires
            # every fp32r-matmul input to be PRODUCED with f32r output dtype,
            # so ones gets a rounded copy and the pair-3 loss DMAs declare
            # their SBUF output APs as f32r (same bits; non-matmul readers
            # keep reading the tile as plain f32).
            ones_r = singles.tile([128, 1], F32R)
            nc.vector.tensor_copy(ones_r, ones)
            # accum slots are written (not accumulated) by accum_out for the
            # partitions each op covers; rows past hsz keep this zero fill.
            acc = singles.tile([128, OUTC], F32)
            nc.vector.memset(acc, 0.0)
            # acc2 cols: 0..3 fg (b6C, b7B, b7C, b7A-chain), 4 close_b, 5 b7-sall
            acc2 = singles.tile([128, 6], F32)
            nc.vector.memset(acc2, 0.0)
            # PSUM prefix accumulators for the PE-side all-sums: ps_a
            # collects pair-3 hc0/hc1 (closed mid-stream by Act), ps_b the
            # tail pieces (closed by Act at the end). SEPARATE tiles: on a
            # shared tile ps_b's matmuls pick up a false WAR against the
            # close_a read. 1 bank each + cpsum's 2x3 = 8.
            ps_a = spsum.tile([1, 256], F32, tag="psa")
            ps_b = spsum.tile([1, 256], F32, tag="psb")

            # batch-in-pair q lives at partition base 64*q (matmul requires
            # operand base partitions of 0/32/64); partitions 48..63 are
            # zeroed padding.
            NP = 64 + N  # 112 partitions spanned by the two batches

            # All boxes in two DMAs, already in the (q*64+n) partition layout
            # used by the mask builds: bx_all[q*64+n, 4*p+c] = boxes[2p+q, n, c].
            bx_all = singles.tile([128, 4 * PAIRS], F32)
            nc.vector.memset(bx_all, 0.0)
            for q in range(2):
                nc.sync.dma_start(
                    out=bx_all[64 * q : 64 * q + N, :],
                    in_=boxes_d[q],
                )
            # (u1-1, v1-1) per box-instance, all pairs in one op
            bm1_all = singles.tile([128, 2 * PAIRS], F32)
            nc.vector.tensor_scalar(
                bm1_all[:NP].rearrange("n (p c) -> n p c", p=PAIRS),
                bx_all[:NP].rearrange("n (p c) -> n p c", p=PAIRS)[:, :, 0:2],
                1.0,
                None,
                OP.subtract,
            )

            def build_masks(p, row_eng, col_eng):
                bx = bx_all[:, 4 * p : 4 * (p + 1)]
                bm1 = bm1_all[:, 2 * p : 2 * (p + 1)]
                # rows: (h > v1-1) & (h < v2)  as fp16
                rowa = masks.tile([128, H], FP16, tag="rowa")
                row_eng.tensor_scalar(
                    rowa[:NP], iota_f[:NP, :H], bm1[:NP, 1:2], None, OP.is_gt
                )
                rowb = masks.tile([128, H], FP16, tag="rowb")
                row_eng.tensor_scalar(
                    rowb[:NP], iota_f[:NP, :H], bx[:NP, 3:4], None, OP.is_lt
                )
                rowm = masks.tile([128, H], FP16, tag="rowm")
                row_eng.tensor_tensor(rowm[:NP], rowa[:NP], rowb[:NP], OP.mult)
                # cols: (w > u1-1) & (w < u2)  as fp16
                cola = masks.tile([128, W], FP16, tag="cola")
                col_eng.tensor_scalar(
                    cola[:NP], iota_f[:NP, :], bm1[:NP, 0:1], None, OP.is_gt
                )
                colb = masks.tile([128, W], FP16, tag="colb")
                col_eng.tensor_scalar(
                    colb[:NP], iota_f[:NP, :], bx[:NP, 2:3], None, OP.is_lt
                )
                colm = masks.tile([128, W], FP16, tag="colm")
                col_eng.tensor_tensor(colm[:NP], cola[:NP], colb[:NP], OP.mult)
                return rowm, colm

            # Pool builds masks serially (its multiply runs at 0.42
            # efficiency), so it gets p1 and p2 plus p3's cheap rows, in an
            # order that has each pair ready before its chunks arrive; DVE
            # takes p0 and p3's wide column masks (fp16 4x mode). p3's masks
            # finish ~17us in, so the pair-3 mask preps can run mid-stream.
            prebuilt = [None] * PAIRS
            prebuilt[0] = build_masks(0, nc.vector, nc.vector)
            prebuilt[1] = build_masks(1, nc.gpsimd, nc.gpsimd)
            prebuilt[3] = build_masks(3, nc.gpsimd, nc.vector)
            prebuilt[2] = build_masks(2, nc.gpsimd, nc.gpsimd)

            def emit_cnt(rowm, colm, q, h0, hsz):
                cnt = cpsum.tile([128, W], F32, tag="cnt")
                for w0 in range(0, W, 512):
                    wsz = min(512, W - w0)
                    nc.tensor.matmul(
                        cnt[:hsz, w0 : w0 + wsz],
                        lhsT=rowm[64 * q : 64 * q + N, h0 : h0 + hsz],
                        rhs=colm[64 * q : 64 * q + N, w0 : w0 + wsz],
                        start=True,
                        stop=True,
                    )
                return cnt

            def fg_stt(eng, lt, cnt, hsz, q, c0, csz, col):
                # fused (mask > 0) * loss with free per-partition row sums;
                # cols >= 26 are late writers and live in acc2
                ac, c = (acc2, col - 26) if col >= 26 else (acc, col)
                sf = sfpool.tile([128, W], F32, tag="sf")
                eng.scalar_tensor_tensor(
                    sf[:hsz, c0 : c0 + csz],
                    cnt[:hsz, c0 : c0 + csz],
                    0.0,
                    lt[:hsz, q * W + c0 : q * W + c0 + csz],
                    op0=OP.is_gt,
                    op1=OP.mult,
                    accum_out=ac[:hsz, c : c + 1],
                )

            def act_sall(lt, hsz, f0, fsz, col):
                ac, c = (acc2, col - 26) if col in (29,) else (acc, col)
                sa = scratch.tile([128, 2 * W], F32, tag="sa")
                nc.scalar.activation(
                    out=sa[:hsz, :fsz],
                    in_=lt[:hsz, f0 : f0 + fsz],
                    func=AF.Copy,
                    accum_out=ac[:hsz, c : c + 1],
                )

            # PE all-sum: ones^T @ loss-piece (f32r), accumulating into the
            # [0:n] prefix of a psum tile across many pieces.
            pe_first = {"psa": True, "psb": True}

            def pe_sall(ps, key, lt, hsz, f0, csz, pfx, last=False):
                # accumulate sum over lt[:hsz, f0:f0+csz] into ps[0:1, 0:pfx]
                for cc in range(0, csz, pfx):
                    n = min(pfx, csz - cc)
                    nc.tensor.matmul(
                        ps[0:1, 0:n],
                        lhsT=ones_r[:hsz, 0:1],
                        rhs=lt[:hsz, f0 + cc : f0 + cc + n].bitcast(F32R),
                        start=pe_first[key],
                        stop=last and cc + pfx >= csz,
                        skip_group_check=True,
                    )
                    pe_first[key] = False

            # ---------------- pairs 0-2: steady state -------------------
            # Pair-3 fg masks are precomputed into SBUF as fp16 0/1 tiles
            # via Act Sign passes (GPSIMD cannot read PSUM, and counts >= 0
            # so sign(cnt) is exactly the fg mask), one per pair-1/2 chunk
            # slot. Each prep's PSUM cnt-buffer hold fits in the cpsum
            # rotation between neighbouring chunks' counts.
            # One 512-col prep sub-pass per chunk slot, in a dedicated
            # one-bank PSUM pool: a prep cnt inside the cpsum rotation makes
            # cnt(k) depend on stt(k-1) instead of stt(k-2), serializing the
            # whole DVE stt chain ~600ns per prep slot.
            # Pair-3 fg masks hc0q1/t6/t7 are precomputed as fp16 Sign
            # tiles from full-width PSUM count tiles inside the cpsum
            # rotation, one per pair-1/2 chunk slot (counts >= 0, so
            # sign(cnt) is exactly the fg mask).
            sgn = {}
            prep_specs = {
                (1, 0): [("t6", 2, 0)],
                (1, 1): [("t7", 2, 1)],
                (1, 2): [("hc0q1", 0, 1)],
            }

            def emit_prep(key, hct, qt):
                rowm3, colm3 = prebuilt[3]
                h0t, hszt = H_CHUNKS[hct]
                cntp = emit_cnt(rowm3, colm3, qt, h0t, hszt)
                csb = singles.tile([128, W], FP16, tag=f"sgn{key}")
                nc.scalar.activation(out=csb[:hszt], in_=cntp[:hszt], func=AF.Sign)
                sgn[key] = csb

            for p in range(3):
                rowm, colm = prebuilt[p]
                for hc, (h0, hsz) in enumerate(H_CHUNKS):
                    lt = ltiles.tile([128, 2 * W], F32, tag="lt")
                    nc.sync.dma_start(
                        out=lt[:hsz].rearrange("h (b w) -> h b w", b=2),
                        in_=loss_d[2 * p : 2 * p + 2, h0 : h0 + hsz, :].rearrange(
                            "b h w -> h b w"
                        ),
                    )
                    for q in range(2):
                        cnt = emit_cnt(rowm, colm, q, h0, hsz)
                        fg_stt(nc.vector, lt, cnt, hsz, q, 0, W, p * 6 + hc * 2 + q)
                    # prep BEFORE the slot's sall: the prep's PSUM buffer
                    # returns to the rotation only after Act runs the Sign.
                    for key, hct, qt in prep_specs.get((p, hc), []):
                        emit_prep(key, hct, qt)
                    act_sall(lt, hsz, 0, 2 * W, 30 + p * 3 + hc)

            # ---------------- pair 3: balanced endgame ------------------
            # hc0/hc1: one DMA per batch; fg stts read the prebuilt sign
            # masks (hc0q1 via the Pool-product -> Act-sall chain, the rest
            # on DVE); all-sums on PE -> ps_a.
            def fg_chain(lt, key, hsz, q, c0, csz, col):
                # Pool mask*loss product, then Act sums it into acc (GPSIMD
                # has no accumulator and cannot read PSUM)
                mlt = sfpool.tile([128, W], F32, tag="mlt")
                nc.gpsimd.tensor_tensor(
                    mlt[:hsz, 0:csz],
                    sgn[key][:hsz, c0 : c0 + csz],
                    lt[:hsz, q * W + c0 : q * W + c0 + csz],
                    OP.mult,
                )
                act_sall(mlt, hsz, 0, csz, col)

            rowm3, colm3 = prebuilt[3]
            hc_specs = [
                (0, 0, "D", 18),
                (1, 0, "C", 19),
                (0, 1, "D", 20),
                (1, 1, "D", 21),
            ]
            hts = {}
            for q, hct, eng, col in hc_specs:
                h0, hsz = H_CHUNKS[hct]
                ht = hts.get(hct)
                if ht is None:
                    ht = ltiles.tile([128, 2 * W], F32, tag="lt")
                    hts[hct] = ht
                nc.sync.dma_start(
                    out=ht[:hsz, q * W : (q + 1) * W].bitcast(F32R),
                    in_=loss_d[6 + q, h0 : h0 + hsz, :].bitcast(F32R),
                )
                pe_sall(ps_a, "psa", ht, hsz, q * W, W, 256,
                        last=(q, hct) == (1, 1))
                if eng == "C":
                    fg_chain(ht, "hc0q1", hsz, q, 0, W, col)
                else:
                    cnt = emit_cnt(rowm3, colm3, q, h0, hsz)
                    fg_stt(nc.vector, ht, cnt, hsz, q, 0, W, col)
            # close ps_a into acc col 37 (Act is idle here)
            cls_a = scratch.tile([128, 2 * W], F32, tag="sa")
            nc.scalar.activation(
                out=cls_a[0:1, 0:256],
                in_=ps_a[0:1, 0:256],
                func=AF.Copy,
                accum_out=acc[0:1, 39:40],
            )

            # tail chunk (120 rows): six pieces, fg on DVE (plus one Pool
            # chain) per the makespan search; all-sums on PE -> ps_b as
            # [1,>=256] f32r chunks decoupled from the piece boundaries
            # (each chunk fires once the piece covering its end arrives),
            # closed by Act at the very end.
            h0, hsz = H_CHUNKS[2]
            lt2 = ltiles.tile([128, 2 * W], F32, tag="lt")
            fgms = (sgn["t6"], sgn["t7"])
            # b7's [992:1248] all-sum chunk is NOT in ps_b: it would gate the
            # Act close at last-byte+~1.9us; instead DVE sums it right after
            # its last fg op (col 40) and the close fires off the b6-gated
            # second-latest chunk.
            chunks = {
                0: list(TAIL_SALL_CHUNKS),
                1: list(TAIL_SALL_CHUNKS[:-1]),
            }
            emitted = {0: 0, 1: 0}  # per batch: next sall chunk index
            n_chunks = len(chunks[0]) + len(chunks[1])
            n_done = 0
            for i, (q, c0, csz, eng, col) in enumerate(TAIL_PIECES):
                nc.sync.dma_start(
                    out=lt2[:hsz, q * W + c0 : q * W + c0 + csz].bitcast(F32R),
                    in_=loss_d[6 + q, h0 : h0 + hsz, c0 : c0 + csz].bitcast(F32R),
                )
                while emitted[q] < len(chunks[q]):
                    s0, ssz = chunks[q][emitted[q]]
                    if s0 + ssz > c0 + csz:
                        break
                    emitted[q] += 1
                    n_done += 1
                    pe_sall(ps_b, "psb", lt2, hsz, q * W + s0, ssz, 256,
                            last=n_done == n_chunks)
                if eng == "C":
                    fg_chain(lt2, f"t{6 + q}", hsz, q, c0, csz, col)
                elif i == 1:
                    # b7A: first 384 cols via a second Pool->Act chain, the
                    # rest on DVE — frees DVE for the late pieces
                    fg_chain(lt2, "t7", hsz, q, 0, 384, 29)
                    fg_stt(nc.vector, lt2, fgms[q], hsz, q, 384, csz - 384, col)
                else:
                    fg_stt(nc.vector, lt2, fgms[q], hsz, q, c0, csz, col)
            # b7 [992:1248] all-sum on DVE, queued right after its last fg
            sfd = sfpool.tile([128, W], F32, tag="sf")
            nc.vector.tensor_scalar(
                sfd[:hsz, 0:256],
                lt2[:hsz, W + 992 : W + 1248],
                1.0,
                0.0,
                OP.mult,
                OP.add,
                accum_out=acc2[:hsz, 5:6],
            )
            # close ps_b into acc col 39 on Act (its queue is otherwise
            # drained by now; DVE is still finishing the last fg stts)
            cls_b = scratch.tile([128, 2 * W], F32, tag="sa")
            nc.scalar.activation(
                out=cls_b[0:1, 0:256],
                in_=ps_b[0:1, 0:256],
                func=AF.Copy,
                accum_out=acc2[0:1, 4:5],
            )

            nc.sync.dma_start(out=out_d[:, :], in_=acc)
            nc.sync.dma_start(out=out2_d[:, :], in_=acc2)

    nc.finalize()
    return nc


def get_nc():
    global _NC_CACHE
    if _NC_CACHE is None:
        _NC_CACHE = _build_nc()
    return _NC_CACHE


def run_cores(loss, gt_boxes2d, trace=False, **kw):
    loss = np.ascontiguousarray(loss, dtype=np.float32)
    boxes = np.ascontiguousarray(gt_boxes2d, dtype=np.float32)
    in_maps = []
    for c in range(N_CORES):
        bc = boxes[c * BPC : (c + 1) * BPC]  # [BPC, N, 4]
        # host-side permutation to the kernel's (q, n, p, c) layout
        bt = bc.reshape(PAIRS, 2, N, 4).transpose(1, 2, 0, 3)
        in_maps.append(
            {
                "loss": np.ascontiguousarray(loss[c * BPC : (c + 1) * BPC]),
                "boxes": np.ascontiguousarray(bt.reshape(2, N, 4 * PAIRS)),
            }
        )
    return run_bass_kernel_spmd(
        get_nc(), in_maps, core_ids=list(range(N_CORES)), trace=trace, **kw
    )


def kernel(loss, gt_boxes2d):
    res = run_cores(loss, gt_boxes2d)
    s_fg = 0.0
    s_all = 0.0
    for r in res.results:
        o = np.asarray(r["out"], dtype=np.float64)
        o2 = np.asarray(r["out2"], dtype=np.float64)
        s_fg += float(o[:, FG_LO:FG_HI].sum()) + float(o2[:, 0:4].sum())
        s_all += float(o[:, ALL_LO:ALL_HI].sum()) + float(o2[:, 4:6].sum())
    n_pix = float(B * H * W)
    fg_loss = FG_WEIGHT * s_fg / n_pix
    bg_loss = (s_all - s_fg) / n_pix
    total = fg_loss + bg_loss
    return (
        np.array(total, dtype=np.float32),
        np.array(fg_loss, dtype=np.float32),
        np.array(bg_loss, dtype=np.float32),
    )


# revision 38
# speedup vs baseline: 1.0001x; 1.0001x over previous
"""Trainium2 Bass kernel for nn_Balancer_10660108829428.

Computes (total, fg_loss, bg_loss) for a fg/bg-weighted loss balancer:
  fg_mask[b,h,w] = any of 48 boxes covers pixel (h,w)
  fg_loss = 13 * sum(loss * fg) / (B*H*W)
  bg_loss = sum(loss * ~fg) / (B*H*W)
  total   = fg_loss + bg_loss

Strategy: data-parallel over B across 8 NeuronCores (8 batches each).
Per core each batch's mask is rasterized as a rank-48 matmul
(row_inT @ col_in) on the TensorEngine; fused DVE scalar_tensor_tensor
ops compute (mask > 0) * loss with a free per-partition row-sum
accumulator.

The kernel is DMA-bound (15 MB of loss per core at 360 GB/s in the
cost model, serialized on the single DMA_ENGINES device), so the whole
game is the head (first-DMA issue latency, ~2.0us) and the tail after
the last loss byte:
  last byte -> +900ns DMA-sem prop to the consumer -> last compute op
  -> out-DMA issue (seq+HWDGE+DGE ~1.3us) -> transfer -> +900ns sem
  prop -> ~0.6us barrier epilogue.
Tail design (engine assignments from an offline makespan search over
the cost model: DVE stt 1.042ns/col+61, Pool product 1.98ns/col+95,
Act sall 0.833ns/col+372+187 accum-read, PE [1,n>=256] f32r sall
107ns/256 cols, every op gated at its piece's arrival + ~947ns):
 - pairs 0-2 run the proven line-rate pipeline: per chunk one 2-batch
   DMA, Act all-sum, two PE count matmuls + DVE stt fg ops; three
   pair-3 fg masks (hc0q1, t6, t7) are precomputed as fp16 Sign tiles
   in the count rotation (sign(cnt) == fg since counts >= 0);
 - pair-3 hc0/hc1: per-batch DMAs; fg on DVE from just-in-time PSUM
   counts except hc0q1 which goes Pool-product -> Act-sall (GPSIMD has
   no accumulator and cannot read PSUM); all-sums on PE -> ps_a,
   closed mid-stream by Act;
 - tail (120 rows): pieces b6 512/224/224/288, b7 736/288/224 (b6's
   middle 448 split into two DMAs purely for finer semaphore grain —
   it pulls two DVE ops into an idle gap); fg on DVE except b6[0:512]
   and b7[0:384] which go through Pool-product -> Act-sall chains;
   all-sums as PE [1,>=256] f32r chunks -> ps_b except b7[992:1248]
   which DVE sums right after its last fg (it would otherwise gate the
   Act close at last-byte + ~1.9us); the Act close of ps_b and DVE's
   last op both land ~1.6us after the last byte, balanced within
   ~100ns, right at the out-DMA issue path's floor;
 - the accumulator is split in two: the bulk [128,42] tile (early
   writers) DMAs out ~1.2us before the end and a tiny [128,6] tile
   holding only the six late-written columns rides a second DMA whose
   transfer runs at the 7ns/descriptor floor — the two issue windows
   don't overlap, so HWDGE never serializes them.
The final reduction of the [128, OUTC] accumulator tile is done on the
host (pure gather arithmetic over named column groups).

Box membership avoids floor/ceil entirely: for integer h,
  h >= floor(v1)  <=>  h > v1 - 1      and      h < ceil(v2)  <=>  h < v2.
"""

import numpy as np

import concourse.bacc as bacc
import concourse.mybir as mybir
import concourse.tile as tile
from concourse.bass_utils import run_bass_kernel_spmd

B, H, W, N = 64, 376, 1248, 48
N_CORES = 8
BPC = B // N_CORES          # batches per core
PAIRS = BPC // 2            # batch pairs per core (masks built 2 batches at a time)
FG_WEIGHT = 13.0
H_CHUNKS = [(0, 128), (128, 128), (256, H - 256)]  # (h0, hsz)
F32 = mybir.dt.float32
F32R = mybir.dt.float32r
BF16 = mybir.dt.bfloat16
FP16 = mybir.dt.float16

# accumulator column layout (host sums these)
#  0..17  : fg partials pairs 0-2 (p*6 + hc*2 + q)
#  18..21 : pair-3 hc0q0, hc0q1, hc1q0, hc1q1 fg
#  22..28 : pair-3 tail piece fg
#  29     : b7A cols [0:384] fg (Pool-product -> Act-sall chain)
#  30..38 : all-pixel partials pairs 0-2 (Act salls, 30 + p*3 + hc)
#  39     : ps_a close (pair-3 hc0/hc1 all-sum)
#  40     : ps_b close (pair-3 tail all-sum, minus b7 [992:1248])
#  41     : b7 [992:1248] all-sum (DVE, after its last fg op)
OUTC = 42
FG_LO, FG_HI = 0, 30
ALL_LO, ALL_HI = 30, 42

# tail piece layout: (batch-in-pair q, c0, csz, fg engine, acc col)
# "C" = Pool tensor_tensor product -> Act sall chain (GPSIMD cannot do
# stt-with-accum, and this keeps DVE free for the late pieces).
TAIL_PIECES = [
    (0, 0, 512, "C", 22),
    (1, 0, 736, "D", 23),
    (0, 512, 224, "D", 24),
    (0, 736, 224, "D", 25),
    (0, 960, 288, "D", 26),
    (1, 736, 288, "D", 27),
    (1, 1024, 224, "D", 28),
]
# all-sum sub-chunks per batch for the tail (PE f32r fast path needs
# n >= 256; the leftover 224 chunk goes first where it is gated early)
TAIL_SALL_CHUNKS = [(0, 224), (224, 256), (480, 256), (736, 256), (992, 256)]

_NC_CACHE = None


def _build_nc():
    # Bacc (not bass.Bass): its finalize() runs the TRN2 wait-legalization
    # passes (move_matmul_waits_to_ldweights / generate_event_semaphores) —
    # the ISA allows only one semaphore wait per instruction.
    nc = bacc.Bacc("TRN2")
    loss_d = nc.dram_tensor("loss", [BPC, H, W], F32, kind="ExternalInput")
    # boxes arrive host-transposed to the on-chip (q*64+n, 4*p+c) layout so
    # the DMA descriptors are contiguous 64 B runs instead of 16 B scatters
    boxes_d = nc.dram_tensor("boxes", [2, N, 4 * PAIRS], F32, kind="ExternalInput")
    out_d = nc.dram_tensor("out", [128, OUTC], F32, kind="ExternalOutput")
    # late-written accumulator columns go to a separate tiny tile/output so
    # the bulk out-DMA can issue ~1.2us earlier and the final DMA's transfer
    # shrinks to the 7ns/descriptor floor (56ns vs 119ns)
    out2_d = nc.dram_tensor("out2", [120, 6], F32, kind="ExternalOutput")

    AX = mybir.AxisListType
    OP = mybir.AluOpType
    AF = mybir.ActivationFunctionType

    with tile.TileContext(nc) as tc:
        with (
            tc.tile_pool(name="singles", bufs=1) as singles,
            tc.tile_pool(name="masks", bufs=4) as masks,
            tc.tile_pool(name="ltiles", bufs=7) as ltiles,
            tc.tile_pool(name="scratch", bufs=2) as scratch,
            tc.tile_pool(name="sfpool", bufs=4) as sfpool,
            tc.tile_pool(name="cpsum", bufs=2, space="PSUM") as cpsum,
            tc.tile_pool(name="spsum", bufs=1, space="PSUM") as spsum,
        ):
            # --- constants ---
            iota_i = singles.tile([128, W], mybir.dt.int32)
            nc.gpsimd.iota(iota_i, pattern=[[1, W]], base=0, channel_multiplier=0)
            # fp16 iota: integers <= 2048 are exact, and 2-byte operands let
            # the DVE mask compares run in 4x mode
            iota_f = singles.tile([128, W], FP16)
            nc.vector.tensor_copy(iota_f, iota_i)
            ones = singles.tile([128, 1], F32)
            nc.vector.memset(ones, 1.0)
            # f32r view for the PE all-sum matmuls: the BIR verifier requires
            # every fp32r-matmul input to be PRODUCED with f32r output dtype,
            # so ones gets a rounded copy and the pair-3 loss DMAs declare
            # their SBUF output APs as f32r (same bits; non-matmul readers
            # keep reading the tile as plain f32).
            ones_r = singles.tile([128, 1], F32R)
            nc.vector.tensor_copy(ones_r, ones)
            # accum slots are written (not accumulated) by accum_out for the
            # partitions each op covers; rows past hsz keep this zero fill.
            acc = singles.tile([128, OUTC], F32)
            nc.vector.memset(acc, 0.0)
            # acc2 cols: 0..3 fg (b6C, b7B, b7C, b7A-chain), 4 close_b, 5 b7-sall
            acc2 = singles.tile([128, 6], F32)
            nc.vector.memset(acc2, 0.0)
            # PSUM prefix accumulators for the PE-side all-sums: ps_a
            # collects pair-3 hc0/hc1 (closed mid-stream by Act), ps_b the
            # tail pieces (closed by Act at the end). SEPARATE tiles: on a
            # shared tile ps_b's matmuls pick up a false WAR against the
            # close_a read. 1 bank each + cpsum's 2x3 = 8.
            ps_a = spsum.tile([1, 256], F32, tag="psa")
            ps_b = spsum.tile([1, 256], F32, tag="psb")

            # batch-in-pair q lives at partition base 64*q (matmul requires
            # operand base partitions of 0/32/64); partitions 48..63 are
            # zeroed padding.
            NP = 64 + N  # 112 partitions spanned by the two batches

            # All boxes in two DMAs, already in the (q*64+n) partition layout
            # used by the mask builds: bx_all[q*64+n, 4*p+c] = boxes[2p+q, n, c].
            bx_all = singles.tile([128, 4 * PAIRS], F32)
            nc.vector.memset(bx_all, 0.0)
            for q in range(2):
                nc.sync.dma_start(
                    out=bx_all[64 * q : 64 * q + N, :],
                    in_=boxes_d[q],
                )
            # (u1-1, v1-1) per box-instance, all pairs in one op
            bm1_all = singles.tile([128, 2 * PAIRS], F32)
            nc.vector.tensor_scalar(
                bm1_all[:NP].rearrange("n (p c) -> n p c", p=PAIRS),
                bx_all[:NP].rearrange("n (p c) -> n p c", p=PAIRS)[:, :, 0:2],
                1.0,
                None,
                OP.subtract,
            )

            def build_masks(p, row_eng, col_eng):
                bx = bx_all[:, 4 * p : 4 * (p + 1)]
                bm1 = bm1_all[:, 2 * p : 2 * (p + 1)]
                # rows: (h > v1-1) & (h < v2)  as fp16
                rowa = masks.tile([128, H], FP16, tag="rowa")
                row_eng.tensor_scalar(
                    rowa[:NP], iota_f[:NP, :H], bm1[:NP, 1:2], None, OP.is_gt
                )
                rowb = masks.tile([128, H], FP16, tag="rowb")
                row_eng.tensor_scalar(
                    rowb[:NP], iota_f[:NP, :H], bx[:NP, 3:4], None, OP.is_lt
                )
                rowm = masks.tile([128, H], FP16, tag="rowm")
                row_eng.tensor_tensor(rowm[:NP], rowa[:NP], rowb[:NP], OP.mult)
                # cols: (w > u1-1) & (w < u2)  as fp16
                cola = masks.tile([128, W], FP16, tag="cola")
                col_eng.tensor_scalar(
                    cola[:NP], iota_f[:NP, :], bm1[:NP, 0:1], None, OP.is_gt
                )
                colb = masks.tile([128, W], FP16, tag="colb")
                col_eng.tensor_scalar(
                    colb[:NP], iota_f[:NP, :], bx[:NP, 2:3], None, OP.is_lt
                )
                colm = masks.tile([128, W], FP16, tag="colm")
                col_eng.tensor_tensor(colm[:NP], cola[:NP], colb[:NP], OP.mult)
                return rowm, colm

            # Pool builds masks serially (its multiply runs at 0.42
            # efficiency), so it gets p1 and p2 plus p3's cheap rows, in an
            # order that has each pair ready before its chunks arrive; DVE
            # takes p0 and p3's wide column masks (fp16 4x mode). p3's masks
            # finish ~17us in, so the pair-3 mask preps can run mid-stream.
            prebuilt = [None] * PAIRS
            prebuilt[0] = build_masks(0, nc.vector, nc.vector)
            prebuilt[1] = build_masks(1, nc.gpsimd, nc.gpsimd)
            prebuilt[3] = build_masks(3, nc.gpsimd, nc.vector)
            prebuilt[2] = build_masks(2, nc.gpsimd, nc.gpsimd)

            def emit_cnt(rowm, colm, q, h0, hsz):
                cnt = cpsum.tile([128, W], F32, tag="cnt")
                for w0 in range(0, W, 512):
                    wsz = min(512, W - w0)
                    nc.tensor.matmul(
                        cnt[:hsz, w0 : w0 + wsz],
                        lhsT=rowm[64 * q : 64 * q + N, h0 : h0 + hsz],
                        rhs=colm[64 * q : 64 * q + N, w0 : w0 + wsz],
                        start=True,
                        stop=True,
                    )
                return cnt

            def fg_stt(eng, lt, cnt, hsz, q, c0, csz, col):
                # fused (mask > 0) * loss with free per-partition row sums;
                # cols >= 26 are late writers and live in acc2
                ac, c = (acc2, col - 26) if col >= 26 else (acc, col)
                sf = sfpool.tile([128, W], F32, tag="sf")
                eng.scalar_tensor_tensor(
                    sf[:hsz, c0 : c0 + csz],
                    cnt[:hsz, c0 : c0 + csz],
                    0.0,
                    lt[:hsz, q * W + c0 : q * W + c0 + csz],
                    op0=OP.is_gt,
                    op1=OP.mult,
                    accum_out=ac[:hsz, c : c + 1],
                )

            def act_sall(lt, hsz, f0, fsz, col):
                ac, c = (acc2, col - 26) if col in (29,) else (acc, col)
                sa = scratch.tile([128, 2 * W], F32, tag="sa")
                nc.scalar.activation(
                    out=sa[:hsz, :fsz],
                    in_=lt[:hsz, f0 : f0 + fsz],
                    func=AF.Copy,
                    accum_out=ac[:hsz, c : c + 1],
                )

            # PE all-sum: ones^T @ loss-piece (f32r), accumulating into the
            # [0:n] prefix of a psum tile across many pieces.
            pe_first = {"psa": True, "psb": True}

            def pe_sall(ps, key, lt, hsz, f0, csz, pfx, last=False):
                # accumulate sum over lt[:hsz, f0:f0+csz] into ps[0:1, 0:pfx]
                for cc in range(0, csz, pfx):
                    n = min(pfx, csz - cc)
                    nc.tensor.matmul(
                        ps[0:1, 0:n],
                        lhsT=ones_r[:hsz, 0:1],
                        rhs=lt[:hsz, f0 + cc : f0 + cc + n].bitcast(F32R),
                        start=pe_first[key],
                        stop=last and cc + pfx >= csz,
                        skip_group_check=True,
                    )
                    pe_first[key] = False

            # ---------------- pairs 0-2: steady state -------------------
            # Pair-3 fg masks are precomputed into SBUF as fp16 0/1 tiles
            # via Act Sign passes (GPSIMD cannot read PSUM, and counts >= 0
            # so sign(cnt) is exactly the fg mask), one per pair-1/2 chunk
            # slot. Each prep's PSUM cnt-buffer hold fits in the cpsum
            # rotation between neighbouring chunks' counts.
            # One 512-col prep sub-pass per chunk slot, in a dedicated
            # one-bank PSUM pool: a prep cnt inside the cpsum rotation makes
            # cnt(k) depend on stt(k-1) instead of stt(k-2), serializing the
            # whole DVE stt chain ~600ns per prep slot.
            # Pair-3 fg masks hc0q1/t6/t7 are precomputed as fp16 Sign
            # tiles from full-width PSUM count tiles inside the cpsum
            # rotation, one per pair-1/2 chunk slot (counts >= 0, so
            # sign(cnt) is exactly the fg mask).
            sgn = {}
            prep_specs = {
                (1, 0): [("t6", 2, 0)],
                (1, 1): [("t7", 2, 1)],
                (1, 2): [("hc0q1", 0, 1)],
            }

            def emit_prep(key, hct, qt):
                rowm3, colm3 = prebuilt[3]
                h0t, hszt = H_CHUNKS[hct]
                cntp = emit_cnt(rowm3, colm3, qt, h0t, hszt)
                csb = singles.tile([128, W], FP16, tag=f"sgn{key}")
                nc.scalar.activation(out=csb[:hszt], in_=cntp[:hszt], func=AF.Sign)
                sgn[key] = csb

            for p in range(3):
                rowm, colm = prebuilt[p]
                for hc, (h0, hsz) in enumerate(H_CHUNKS):
                    lt = ltiles.tile([128, 2 * W], F32, tag="lt")
                    nc.sync.dma_start(
                        out=lt[:hsz].rearrange("h (b w) -> h b w", b=2),
                        in_=loss_d[2 * p : 2 * p + 2, h0 : h0 + hsz, :].rearrange(
                            "b h w -> h b w"
                        ),
                    )
                    for q in range(2):
                        cnt = emit_cnt(rowm, colm, q, h0, hsz)
                        fg_stt(nc.vector, lt, cnt, hsz, q, 0, W, p * 6 + hc * 2 + q)
                    # prep BEFORE the slot's sall: the prep's PSUM buffer
                    # returns to the rotation only after Act runs the Sign.
                    for key, hct, qt in prep_specs.get((p, hc), []):
                        emit_prep(key, hct, qt)
                    act_sall(lt, hsz, 0, 2 * W, 30 + p * 3 + hc)

            # ---------------- pair 3: balanced endgame ------------------
            # hc0/hc1: one DMA per batch; fg stts read the prebuilt sign
            # masks (hc0q1 via the Pool-product -> Act-sall chain, the rest
            # on DVE); all-sums on PE -> ps_a.
            def fg_chain(lt, key, hsz, q, c0, csz, col):
                # Pool mask*loss product, then Act sums it into acc (GPSIMD
                # has no accumulator and cannot read PSUM)
                mlt = sfpool.tile([128, W], F32, tag="mlt")
                nc.gpsimd.tensor_tensor(
                    mlt[:hsz, 0:csz],
                    sgn[key][:hsz, c0 : c0 + csz],
                    lt[:hsz, q * W + c0 : q * W + c0 + csz],
                    OP.mult,
                )
                act_sall(mlt, hsz, 0, csz, col)

            rowm3, colm3 = prebuilt[3]
            hc_specs = [
                (0, 0, "D", 18),
                (1, 0, "C", 19),
                (0, 1, "D", 20),
                (1, 1, "D", 21),
            ]
            hts = {}
            for q, hct, eng, col in hc_specs:
                h0, hsz = H_CHUNKS[hct]
                ht = hts.get(hct)
                if ht is None:
                    ht = ltiles.tile([128, 2 * W], F32, tag="lt")
                    hts[hct] = ht
                nc.sync.dma_start(
                    out=ht[:hsz, q * W : (q + 1) * W].bitcast(F32R),
                    in_=loss_d[6 + q, h0 : h0 + hsz, :].bitcast(F32R),
                )
                pe_sall(ps_a, "psa", ht, hsz, q * W, W, 256,
                        last=(q, hct) == (1, 1))
                if eng == "C":
                    fg_chain(ht, "hc0q1", hsz, q, 0, W, col)
                else:
                    cnt = emit_cnt(rowm3, colm3, q, h0, hsz)
                    fg_stt(nc.vector, ht, cnt, hsz, q, 0, W, col)
            # close ps_a into acc col 37 (Act is idle here)
            cls_a = scratch.tile([128, 2 * W], F32, tag="sa")
            nc.scalar.activation(
                out=cls_a[0:1, 0:256],
                in_=ps_a[0:1, 0:256],
                func=AF.Copy,
                accum_out=acc[0:1, 39:40],
            )

            # tail chunk (120 rows): six pieces, fg on DVE (plus one Pool
            # chain) per the makespan search; all-sums on PE -> ps_b as
            # [1,>=256] f32r chunks decoupled from the piece boundaries
            # (each chunk fires once the piece covering its end arrives),
            # closed by Act at the very end.
            h0, hsz = H_CHUNKS[2]
            lt2 = ltiles.tile([128, 2 * W], F32, tag="lt")
            fgms = (sgn["t6"], sgn["t7"])
            # b7's [992:1248] all-sum chunk is NOT in ps_b: it would gate the
            # Act close at last-byte+~1.9us; instead DVE sums it right after
            # its last fg op (col 40) and the close fires off the b6-gated
            # second-latest chunk.
            chunks = {
                0: list(TAIL_SALL_CHUNKS),
                1: list(TAIL_SALL_CHUNKS[:-1]),
            }
            emitted = {0: 0, 1: 0}  # per batch: next sall chunk index
            n_chunks = len(chunks[0]) + len(chunks[1])
            n_done = 0
            for i, (q, c0, csz, eng, col) in enumerate(TAIL_PIECES):
                nc.sync.dma_start(
                    out=lt2[:hsz, q * W + c0 : q * W + c0 + csz].bitcast(F32R),
                    in_=loss_d[6 + q, h0 : h0 + hsz, c0 : c0 + csz].bitcast(F32R),
                )
                while emitted[q] < len(chunks[q]):
                    s0, ssz = chunks[q][emitted[q]]
                    if s0 + ssz > c0 + csz:
                        break
                    emitted[q] += 1
                    n_done += 1
                    pe_sall(ps_b, "psb", lt2, hsz, q * W + s0, ssz, 256,
                            last=n_done == n_chunks)
                if eng == "C":
                    fg_chain(lt2, f"t{6 + q}", hsz, q, c0, csz, col)
                elif i == 1:
                    # b7A: first 384 cols via a second Pool->Act chain, the
                    # rest on DVE — frees DVE for the late pieces
                    fg_chain(lt2, "t7", hsz, q, 0, 384, 29)
                    fg_stt(nc.vector, lt2, fgms[q], hsz, q, 384, csz - 384, col)
                else:
                    fg_stt(nc.vector, lt2, fgms[q], hsz, q, c0, csz, col)
            # b7 [992:1248] all-sum on DVE, queued right after its last fg
            sfd = sfpool.tile([128, W], F32, tag="sf")
            nc.vector.tensor_scalar(
                sfd[:hsz, 0:256],
                lt2[:hsz, W + 992 : W + 1248],
                1.0,
                0.0,
                OP.mult,
                OP.add,
                accum_out=acc2[:hsz, 5:6],
            )
            # close ps_b into acc col 39 on Act (its queue is otherwise
            # drained by now; DVE is still finishing the last fg stts)
            cls_b = scratch.tile([128, 2 * W], F32, tag="sa")
            nc.scalar.activation(
                out=cls_b[0:1, 0:256],
                in_=ps_b[0:1, 0:256],
                func=AF.Copy,
                accum_out=acc2[0:1, 4:5],
            )

            nc.sync.dma_start(out=out_d[:, :], in_=acc)
            # only partitions 0..119 of acc2 are ever written (tail rows +
            # the close's row 0) — fewer descriptors on the terminal DMA
            nc.sync.dma_start(out=out2_d[:, :], in_=acc2[0:120, :])

    nc.finalize()
    return nc


def get_nc():
    global _NC_CACHE
    if _NC_CACHE is None:
        _NC_CACHE = _build_nc()
    return _NC_CACHE


def run_cores(loss, gt_boxes2d, trace=False, **kw):
    loss = np.ascontiguousarray(loss, dtype=np.float32)
    boxes = np.ascontiguousarray(gt_boxes2d, dtype=np.float32)
    in_maps = []
    for c in range(N_CORES):
        bc = boxes[c * BPC : (c + 1) * BPC]  # [BPC, N, 4]
        # host-side permutation to the kernel's (q, n, p, c) layout
        bt = bc.reshape(PAIRS, 2, N, 4).transpose(1, 2, 0, 3)
        in_maps.append(
            {
                "loss": np.ascontiguousarray(loss[c * BPC : (c + 1) * BPC]),
                "boxes": np.ascontiguousarray(bt.reshape(2, N, 4 * PAIRS)),
            }
        )
    return run_bass_kernel_spmd(
        get_nc(), in_maps, core_ids=list(range(N_CORES)), trace=trace, **kw
    )


def kernel(loss, gt_boxes2d):
    res = run_cores(loss, gt_boxes2d)
    s_fg = 0.0
    s_all = 0.0
    for r in res.results:
        o = np.asarray(r["out"], dtype=np.float64)
        o2 = np.asarray(r["out2"], dtype=np.float64)
        s_fg += float(o[:, FG_LO:FG_HI].sum()) + float(o2[:, 0:4].sum())
        s_all += float(o[:, ALL_LO:ALL_HI].sum()) + float(o2[:, 4:6].sum())
    n_pix = float(B * H * W)
    fg_loss = FG_WEIGHT * s_fg / n_pix
    bg_loss = (s_all - s_fg) / n_pix
    total = fg_loss + bg_loss
    return (
        np.array(total, dtype=np.float32),
        np.array(fg_loss, dtype=np.float32),
        np.array(bg_loss, dtype=np.float32),
    )
